# revision 9
# baseline (speedup 1.0000x reference)
"""Trainium2 Bass kernel for ConvolutionalSelfAttention.

Math (per batch image, fp32):
  X [256, 64] pixels.  For each 3x3 window n (196 of them) and local slot k
  (9), the reference softmax-attends over the 247 pixels outside window n
  with logits TEMP*cos(x_g, x_{pix(n,k)}), weights s_g = x_g @ Wg + bg, and
  aggregates the window pixels with the resulting per-slot weights.

  Key factorization: all needed cosine sims live in one 256x256 gram
  E = exp(TEMP * Xn @ Xn.T); window/global masking is linear, so
      D[p, n] = sum_g maskg[g, n] * E[g, p]          (denominator)
      N[p, n] = sum_g maskg[g, n] * s'_g * E[g, p]   (numerator)
      A[p, n] = maskl[p, n] * N[p, n] / D[p, n]
      out[n, c] = sum_p A[p, n] * X[p, c]
  -> everything is dense matmuls + one exp, no per-window gathers.

Sharding: data-parallel over batch; 32 images / 8 cores = 4 images per core.
"""

import sys
import numpy as np
import ml_dtypes

sys.path.insert(0, "/opt/trn_rl_repo")

from contextlib import ExitStack

import concourse.bass as bass
import concourse.bacc as bacc
import concourse.tile as tile
from concourse import mybir
from concourse.bass_utils import run_bass_kernel_spmd

H = 16
W = 16
C = 64
K = 3
B = 32
CH = H - K + 1
CW = W - K + 1
NC = CH * CW          # 196
HW = H * W            # 256
TEMP = 10.0
NCORES = 8
BPC = B // NCORES     # 4 images per core
P = 128

F32 = mybir.dt.float32
BF16 = mybir.dt.bfloat16
AF = mybir.ActivationFunctionType
ALU = mybir.AluOpType


def _masks():
    maskl = np.zeros((HW, NC), np.float32)
    for i in range(CH):
        for j in range(CW):
            n = i * CW + j
            m = np.zeros((H, W), bool)
            m[i:i + K, j:j + K] = True
            maskl[m.reshape(-1), n] = 1.0
    return maskl, (1.0 - maskl).astype(np.float32)


MASKL, MASKG = _masks()
MASKL_BF = MASKL.astype(ml_dtypes.bfloat16)
MASKG_BF = MASKG.astype(ml_dtypes.bfloat16)
IDENT = np.eye(P if (P:=128) else 128, dtype=np.float32)


def _bcast_ap(ap, parts):
    """[*dims] -> [parts, *dims] with partition stride 0 (DMA broadcast)."""
    return bass.AP(tensor=ap.tensor, offset=ap.offset, ap=[[0, parts]] + list(ap.ap))


def _patch_act_tables():
    """Steer every Ln/Exp activation to `natural_log_exp_and_others` so the
    kernel needs exactly one ACT table load instead of thrashing between the
    Ln-only and Exp-only sets (~2.7us per switch)."""
    from concourse import hw_specs
    orig_fn = hw_specs.get_activation_tables.__wrapped__

    def patched(arch):
        tabs = dict(orig_fn(arch))
        if "natural_log_exp_and_others" in tabs:
            for name in tabs:
                if name != "natural_log_exp_and_others":
                    tabs[name] = tabs[name] - {AF.Ln, AF.Exp}
        return tabs

    bacc.get_activation_tables = patched


def build_bass():
    _patch_act_tables()
    nc = bacc.Bacc("TRN2", target_bir_lowering=False, debug=False)

    x = nc.declare_dram_parameter("x", [BPC, HW, C], BF16, isOutput=False)
    wg = nc.declare_dram_parameter("wg", [C, 1], F32, isOutput=False)
    bg = nc.declare_dram_parameter("bg", [1], F32, isOutput=False)
    mgd = nc.declare_dram_parameter("maskg", [HW, NC], BF16, isOutput=False)
    mld = nc.declare_dram_parameter("maskl", [HW, NC], BF16, isOutput=False)
    idd = nc.declare_dram_parameter("ident", [P, P], F32, isOutput=False)
    y = nc.declare_dram_parameter("y", [BPC, NC, C], BF16, isOutput=True)

    with ExitStack() as ctx:
        tc = ctx.enter_context(tile.TileContext(nc))
        consts = ctx.enter_context(tc.tile_pool(name="consts", bufs=1))
        sb = ctx.enter_context(tc.tile_pool(name="sb", bufs=1))
        pt_pool = ctx.enter_context(tc.tile_pool(name="pt", bufs=1, space="PSUM"))
        pg_pool = ctx.enter_context(tc.tile_pool(name="pg", bufs=1, space="PSUM"))
        pnd_pool = ctx.enter_context(tc.tile_pool(name="pnd", bufs=1, space="PSUM"))

        ident = consts.tile([P, P], F32, tag="ident")
        nc.sync.dma_start(out=ident, in_=idd[:, :])
        wb = consts.tile([P, C], F32, tag="wb")
        nc.sync.dma_start(out=wb, in_=_bcast_ap(wg[:, 0], P))
        bgb = consts.tile([P, 1], F32, tag="bgb")
        nc.sync.dma_start(out=bgb, in_=_bcast_ap(bg[:], P))

        mg = []
        ml = []
        for t in range(2):
            mgt = consts.tile([P, NC], BF16, tag=f"mg{t}")
            nc.sync.dma_start(out=mgt, in_=mgd[t * P:(t + 1) * P, :])
            mg.append(mgt)
            mlt = consts.tile([P, NC], BF16, tag=f"ml{t}")
            nc.sync.dma_start(out=mlt, in_=mld[t * P:(t + 1) * P, :])
            ml.append(mlt)

        # ---- stage 1: load, row stats; ACT does only Ln here ----
        xt = [[None] * 2 for _ in range(BPC)]
        sp = [[None] * 2 for _ in range(BPC)]
        uu = [[None] * 2 for _ in range(BPC)]
        for b in range(BPC):
            for t in range(2):
                xbt = sb.tile([P, C], BF16, tag=f"xb{b}{t}")
                nc.sync.dma_start(out=xbt, in_=x[b, t * P:(t + 1) * P, :])
                xtt = sb.tile([P, C], F32, tag=f"x{b}{t}")
                nc.vector.tensor_copy(out=xtt, in_=xbt)
                xt[b][t] = xtt
                scr = sb.tile([P, C], F32, tag=f"scr{b}{t}")
                nc.gpsimd.tensor_mul(out=scr, in0=xtt, in1=xtt)
                ss = sb.tile([P, 1], F32, tag=f"ss{b}{t}")
                nc.vector.reduce_sum(out=ss, in_=scr, axis=mybir.AxisListType.X)
                scr2 = sb.tile([P, C], F32, tag=f"scr2{b}{t}")
                nc.gpsimd.tensor_mul(out=scr2, in0=xtt, in1=wb)
                s0 = sb.tile([P, 1], F32, tag=f"s0{b}{t}")
                nc.vector.reduce_sum(out=s0, in_=scr2, axis=mybir.AxisListType.X)
                spt = sb.tile([P, 1], F32, tag=f"sp{b}{t}")
                nc.vector.tensor_scalar_add(out=spt, in0=s0, scalar1=bgb[:, 0:1])
                sp[b][t] = spt
                u = sb.tile([P, 1], F32, tag=f"u{b}{t}")
                nc.scalar.activation(out=u, in_=ss, func=AF.Ln)
                uu[b][t] = u

        # ---- stage 2: normalize, transpose, gram, E = exp ----
        e = [[None] * 2 for _ in range(BPC)]
        for b in range(BPC):
            xn = []
            for t in range(2):
                rn = sb.tile([P, 1], F32, tag=f"rn{b}{t}")
                nc.scalar.activation(out=rn, in_=uu[b][t], func=AF.Exp, scale=-0.5)
                xnt = sb.tile([P, C], F32, tag=f"xn{b}{t}")
                nc.vector.tensor_scalar_mul(out=xnt, in0=xt[b][t], scalar1=rn)
                xn.append(xnt)
            xnT = sb.tile([C, HW], F32, tag=f"xnT{b}")
            for t in range(2):
                tp = pt_pool.tile([C, P], F32, tag=f"tp{t}")
                nc.tensor.transpose(out=tp, in_=xn[t], identity=ident)
                nc.vector.tensor_copy(out=xnT[:, t * P:(t + 1) * P], in_=tp)
            for t in range(2):
                g = pg_pool.tile([P, HW], F32, tag=f"g{t}")
                nc.tensor.matmul(
                    out=g, lhsT=xnT[:, t * P:(t + 1) * P], rhs=xnT,
                    start=True, stop=True)
                et = sb.tile([P, HW], BF16, tag=f"e{b}{t}")
                nc.scalar.activation(out=et, in_=g, func=AF.Exp, scale=TEMP)
                e[b][t] = et

        # ---- stage 3: N/D matmuls (bf16 in, f32 psum); ACT: Ln(D) ----
        u2 = [[None] * 2 for _ in range(BPC)]
        nps = [[None] * 2 for _ in range(BPC)]
        for b in range(BPC):
            ms = []
            for t in range(2):
                mst = sb.tile([P, NC], BF16, tag=f"ms{b}{t}")
                nc.vector.tensor_scalar_mul(out=mst, in0=mg[t], scalar1=sp[b][t])
                ms.append(mst)
            for pti in range(2):
                psl = slice(pti * P, (pti + 1) * P)
                d_ps = pnd_pool.tile([P, NC], F32, tag=f"d{pti}")
                nc.tensor.matmul(out=d_ps, lhsT=e[b][0][:, psl], rhs=mg[0],
                                 start=True, stop=False)
                nc.tensor.matmul(out=d_ps, lhsT=e[b][1][:, psl], rhs=mg[1],
                                 start=False, stop=True)
                n_ps = pnd_pool.tile([P, NC], F32, tag=f"n{pti}")
                nc.tensor.matmul(out=n_ps, lhsT=e[b][0][:, psl], rhs=ms[0],
                                 start=True, stop=False)
                nc.tensor.matmul(out=n_ps, lhsT=e[b][1][:, psl], rhs=ms[1],
                                 start=False, stop=True)
                u2t = sb.tile([P, NC], F32, tag=f"u2{b}{pti}")
                nc.scalar.activation(out=u2t, in_=d_ps, func=AF.Ln)
                u2[b][pti] = u2t
                nsb = sb.tile([P, NC], F32, tag=f"nsb{b}{pti}")
                nc.vector.tensor_copy(out=nsb, in_=n_ps)
                nps[b][pti] = nsb

        # ---- stage 4: A = maskl * N * exp(-lnD); out = A.T @ X ----
        for b in range(BPC):
            a = []
            for pti in range(2):
                rd = sb.tile([P, NC], F32, tag=f"rd{b}{pti}")
                nc.scalar.activation(out=rd, in_=u2[b][pti], func=AF.Exp,
                                     scale=-1.0)
                a1 = sb.tile([P, NC], F32, tag=f"a1{b}{pti}")
                nc.vector.tensor_mul(out=a1, in0=nps[b][pti], in1=rd)
                a2 = sb.tile([P, NC], F32, tag=f"a2{b}{pti}")
                nc.gpsimd.tensor_mul(out=a2, in0=a1, in1=ml[pti])
                a.append(a2)
            for nt, (n0, nsz) in enumerate(((0, P), (P, NC - P))):
                o = pg_pool.tile([P, C], F32, tag=f"g{nt}")
                nc.tensor.matmul(out=o[:nsz, :], lhsT=a[0][:, n0:n0 + nsz],
                                 rhs=xt[b][0], start=True, stop=False)
                nc.tensor.matmul(out=o[:nsz, :], lhsT=a[1][:, n0:n0 + nsz],
                                 rhs=xt[b][1], start=False, stop=True)
                osb = sb.tile([P, C], BF16, tag=f"osb{b}{nt}")
                nc.vector.tensor_copy(out=osb[:nsz, :], in_=o[:nsz, :])
                nc.sync.dma_start(out=y[b, n0:n0 + nsz, :], in_=osb[:nsz, :])

    nc.compile()
    return nc


_NC_CACHE = None


def _get_nc():
    global _NC_CACHE
    if _NC_CACHE is None:
        _NC_CACHE = build_bass()
    return _NC_CACHE


class _Runner:
    """Persistent jitted PJRT executable for the Bass program.

    run_bass_kernel_spmd re-creates its jax.jit closure on every call, so each
    invocation pays a full retrace + XLA lowering (~350ms). Here the sharded
    jitted function is built once; warm calls only pay input upload + device
    exec + output download. Constant operands (masks, identity) are placed on
    device once as committed sharded arrays so they are never re-uploaded.
    """

    def __init__(self, nc):
        import jax
        from jax.sharding import Mesh, PartitionSpec, NamedSharding
        from jax.experimental.shard_map import shard_map
        from concourse import bass2jax

        bass2jax.install_neuronx_cc_hook()
        assert nc.dbg_addr is None, "build with debug=False"

        partition_name = (
            nc.partition_id_tensor.name if nc.partition_id_tensor else None
        )
        in_names = []
        out_names = []
        out_avals = []
        zero_shapes = []
        for alloc in nc.m.functions[0].allocations:
            if not isinstance(alloc, mybir.MemoryLocationSet):
                continue
            name = alloc.memorylocations[0].name
            if alloc.kind == "ExternalInput":
                if name != partition_name:
                    in_names.append(name)
            elif alloc.kind == "ExternalOutput":
                shape = tuple(alloc.tensor_shape)
                dtype = mybir.dt.np(alloc.dtype)
                out_names.append(name)
                out_avals.append(jax.core.ShapedArray(shape, dtype))
                zero_shapes.append(((NCORES * shape[0],) + shape[1:], dtype))
        n_params = len(in_names)
        n_outs = len(out_avals)
        all_in_names = list(in_names) + list(out_names)
        if partition_name is not None:
            all_in_names.append(partition_name)
        donate = tuple(range(n_params, n_params + n_outs))

        def _body(*args):
            operands = list(args)
            if partition_name is not None:
                operands.append(bass2jax.partition_id_tensor())
            outs = bass2jax._bass_exec_p.bind(
                *operands,
                out_avals=tuple(out_avals),
                in_names=tuple(all_in_names),
                out_names=tuple(out_names),
                lowering_input_output_aliases=(),
                sim_require_finite=True,
                sim_require_nnan=True,
                nc=nc,
            )
            return tuple(outs)

        devices = jax.devices()[:NCORES]
        mesh = Mesh(np.asarray(devices), ("core",))
        in_specs = (PartitionSpec("core"),) * (n_params + n_outs)
        out_specs = (PartitionSpec("core"),) * n_outs
        self.fn = jax.jit(
            shard_map(
                _body, mesh=mesh, in_specs=in_specs, out_specs=out_specs,
                check_rep=False,
            ),
            donate_argnums=donate,
            keep_unused=True,
        )
        self.in_names = in_names
        self.zero_shapes = zero_shapes
        self.sharding = NamedSharding(mesh, PartitionSpec("core"))
        self.jax = jax
        # Constant operands, replicated per core and committed to device once.
        self.consts = {
            "maskg": jax.device_put(
                np.concatenate([MASKG_BF] * NCORES, 0), self.sharding),
            "maskl": jax.device_put(
                np.concatenate([MASKL_BF] * NCORES, 0), self.sharding),
            "ident": jax.device_put(
                np.concatenate([IDENT] * NCORES, 0), self.sharding),
        }
        # Donation chain: the previous call's output array (already on
        # device) is donated as the next call's output buffer, so no fresh
        # zero buffer is uploaded per call. The kernel writes every output
        # element, so the buffer's stale contents are never observed.
        self._donate_next = None
        self._compiled = None

    def _get_compiled(self, args, bufs):
        if self._compiled is None:
            self._compiled = self.fn.lower(*args, *bufs).compile()
        return self._compiled

    def __call__(self, named_inputs: dict) -> np.ndarray:
        args = [
            self.consts[n] if n in self.consts else named_inputs[n]
            for n in self.in_names
        ]
        if self._donate_next is None:
            bufs = [np.zeros(s, d) for s, d in self.zero_shapes]
        else:
            bufs = [self._donate_next]
        try:
            outs = self._get_compiled(args, bufs)(*args, *bufs)
        except Exception:
            # e.g. a stale donated buffer after an interrupted call —
            # retry once with fresh zero buffers.
            self._donate_next = None
            bufs = [np.zeros(s, d) for s, d in self.zero_shapes]
            outs = self._get_compiled(args, bufs)(*args, *bufs)
        y = np.asarray(outs[0])
        self._donate_next = outs[0]
        return y


_RUNNER_CACHE = None


def _get_runner():
    global _RUNNER_CACHE
    if _RUNNER_CACHE is None:
        _RUNNER_CACHE = _Runner(_get_nc())
    return _RUNNER_CACHE


def kernel(batch: np.ndarray, Wg: np.ndarray, bg: np.ndarray) -> np.ndarray:
    X = np.asarray(batch).reshape(B, HW, C).astype(ml_dtypes.bfloat16)
    wgf = np.ascontiguousarray(np.asarray(Wg, np.float32))
    bgf = np.ascontiguousarray(np.asarray(bg, np.float32))

    run = _get_runner()
    out = run({
        "x": X,
        "wg": np.concatenate([wgf] * NCORES, 0),
        "bg": np.concatenate([bgf] * NCORES, 0),
    })
    return out.reshape(B, CH, CW, C).astype(np.float32)

